# revision 35
# baseline (speedup 1.0000x reference)
"""GCN (3-layer, 3-head) Trainium2 kernel, node-sharded across 8 NeuronCores.

Strategy (per core c of 8):
  - Core c owns nodes [c*6250, (c+1)*6250), padded to 6272 = 49*128.
  - Per layer: lin on own nodes (PE), scale by dinv -> h'' (bf16), AllGather
    h'' into a shared [50176, D] table, then aggregate own-dst edges:
    gather h''[src] rows via dma_gather + 0/1-indicator matmuls accumulating
    per-128-dst-window PSUM tiles; self term added via identity matmul;
    post-scale by dinv[dst], add bias, relu.
  - Edge norms are folded: norm_edge = dinv[src]*dinv[dst] becomes pre-scale
    (h'' = h' * dinv) + post-scale (out *= dinv[dst]); indicator is exact 0/1.
  - Indicators are generated on-chip (DVE iota+is_equal from a compact
    [128, totch] wdst int16 input; -1 pads match nothing) instead of being
    streamed from HBM (-43MB/core); dinv scalings run on the ACT engine to
    keep DVE headroom for indicator generation.
  - Edges are sorted by (dst-window, src-half, src-row) on the host: the
    src-row secondary sort makes gather descriptor addresses monotone per
    call (HBM locality). The shared table is split into two halves (A:
    own-rows [0,3200), B: [3200,6272) of every core) so gather indices fit
    int16 and the two AllGathers pipeline: AG_A fires mid-lin and the A-half
    gathers overlap AG_B's transfer.
  - Message tables for layers 1-2 are fp8e4m3 (halves gather + AllGather
    bytes; end-to-end rel-err ~4.4e-3 vs the 2e-2 gate). Layer 3 stays bf16
    because its 128-wide rows would be 128B in fp8, below the 256B
    dma_gather row minimum. The self-term path stays bf16 throughout.
  - Biases fold into the aggregation PSUM chain as rank-1 matmuls
    (q[dst] x b[feat], q = sqrt(deg)), so the post-path is a single fused
    scale+relu activation.
  - dma_gather descriptor generation is spread over all 4 SWDGE queues.
"""

import numpy as np
import ml_dtypes

import concourse.bass as bass
import concourse.bacc as bacc
import concourse.mybir as mybir
import concourse.tile as tile
from concourse.bass_utils import run_bass_kernel_spmd
from concourse.masks import make_identity

NCORES = 8
N = 50000
E = 800000
D_IN = 512
H = 256
H3 = 128
OUT = 9
NPC = N // NCORES          # 6250 nodes per core
W = 49                     # dst windows of 128 (49*128 = 6272)
CP = W * 128               # padded nodes per core = 6272
TBL = NCORES * CP          # padded gather-table rows = 50176
RA = 3200                  # rows-per-core in table A (25 tiles of 128)
RB = CP - RA               # rows-per-core in table B (24 tiles)
MA = RA // 128             # 25 m-tiles in A
TBLA = NCORES * RA         # 25600 (< 32767, int16-safe)
TBLB = NCORES * RB         # 24576
BLK = 6                    # dst windows per gather block
NSWQ = 4                   # SWDGE queues: parallel Q7 descriptor generation
GBUFS = 2                  # gather tile double-buffering depth
IBUFS = 2                  # indicator tile buffering depth
SINGLE_PACKET = False      # dma_gather single_packet mode
AG_DELAY = 2               # blocks between a fused AG's lins and its trigger
FUSE = False               # fuse lin(l+1)+AG(l+1) into agg(l)'s block loop;
                           # measured slightly worse than the sequential
                           # structure (AG cost is HBM-bandwidth contention,
                           # not exposable latency), so off by default
DOUTS = (H, H, H3)         # per-layer output widths
KOS = (D_IN // 128, H // 128, H // 128)  # per-layer contraction subtiles

BF16 = mybir.dt.bfloat16
F32 = mybir.dt.float32
F8 = mybir.dt.float8e4
I16 = mybir.dt.int16
I8 = mybir.dt.int8

# message-table dtype per layer: fp8 halves gather+AllGather bytes; L3 must
# stay bf16 (fp8 rows would be 128B < the 256B dma_gather row minimum).
# measured end-to-end rel-err with L1+L2 fp8 is ~4e-3 (gate 2e-2).
TDTS = (F8, F8, BF16)


def split_excess_waits(nc, max_waits=1):
    """This container's walrus rejects >1 sync wait per instruction: move
    excess waits onto same-engine NOPs inserted immediately before."""
    n_split = 0
    for f in nc.m.functions:
        for bb in f.blocks:
            new_insts = []
            for ins in bb.instructions:
                si = ins.sync_info
                if si is not None and si.on_wait and len(si.on_wait) > max_waits:
                    waits = list(si.on_wait)
                    excess, keep = waits[:-max_waits], waits[-max_waits:]
                    for i in range(0, len(excess), max_waits):
                        nop = mybir.InstNoOp(
                            name=f"{ins.name}-ws{n_split}",
                            engine=ins.engine,
                            sync_info=mybir.SyncInfo(
                                on_wait=excess[i:i + max_waits], on_update=[]
                            ),
                            bass_nofuse=True,
                        )
                        n_split += 1
                        nc.register_instruction(nop)
                        new_insts.append(nop)
                    si.on_wait = keep
                new_insts.append(ins)
            bb.instructions = new_insts
    return n_split


class PatchedBacc(bacc.Bacc):
    def compile(self):
        super().compile()
        split_excess_waits(self)


def _schedule(counts):
    """counts: [NCORES, W, 2] per-(core,window,half) edge counts.
    Returns the static chunk schedule shared by all cores."""
    ch = -(-counts.max(axis=0) // 128)          # [W, 2] chunks per (window, half)
    blocks = []
    off = 0
    for b0 in range(0, W, BLK):
        ws = list(range(b0, min(b0 + BLK, W)))
        lo_counts = [int(ch[w, 0]) for w in ws]
        hi_counts = [int(ch[w, 1]) for w in ws]
        lo = (off, lo_counts)
        off += sum(lo_counts)
        hi = (off, hi_counts)
        off += sum(hi_counts)
        blocks.append({"windows": ws, "lo": lo, "hi": hi})
    return {"ch": ch, "blocks": blocks, "totch": off}


def _chunk_base(sched):
    """chunk id of the first chunk of each (window, half)."""
    base = np.zeros((W, 2), np.int64)
    for blk in sched["blocks"]:
        for half, key in ((0, "lo"), (1, "hi")):
            off, cnts = blk[key]
            for w, cnt in zip(blk["windows"], cnts):
                base[w, half] = off
                off += cnt
    return base


def _preprocess(x, edge_index):
    src = edge_index[0].astype(np.int64)
    dst = edge_index[1].astype(np.int64)
    deg = np.bincount(dst, minlength=N).astype(np.float32) + 1.0
    dinv = 1.0 / np.sqrt(deg)

    oc = dst // NPC
    ldst = dst - oc * NPC
    win = ldst >> 7
    wdst = ldst & 127
    g = src // NPC
    r = src % NPC
    half = (r >= RA).astype(np.int64)
    rel = np.where(half == 0, g * RA + r, g * RB + (r - RA)).astype(np.int16)

    key = (oc * W + win) * 2 + half
    counts = np.bincount(key, minlength=NCORES * W * 2).reshape(NCORES, W, 2)
    sched = _schedule(counts)
    totch = sched["totch"]
    cbase = _chunk_base(sched)

    # secondary sort by table address (rel) within each group: gather
    # descriptors then read monotonically increasing addresses -> better
    # HBM row locality on the 512B random reads
    order = np.lexsort((rel, key))
    key_s = key[order]
    rel_s = rel[order]
    wdst_s = wdst[order]
    win_s = win[order]
    half_s = half[order]
    oc_s = oc[order]
    # offset of each edge within its (core, window, half) group
    group_start = np.zeros(NCORES * W * 2 + 1, np.int64)
    np.cumsum(counts.reshape(-1), out=group_start[1:])
    off_in_group = np.arange(E) - group_start[key_s]
    chunk_id = cbase[win_s, half_s] + (off_in_group >> 7)
    slot = off_in_group & 127

    per_core = []
    core_edge_start = np.searchsorted(oc_s, np.arange(NCORES + 1))
    for c in range(NCORES):
        s, e = core_edge_start[c], core_edge_start[c + 1]
        idx16 = np.zeros((totch, 128), np.int16)
        idx16[chunk_id[s:e], slot[s:e]] = rel_s[s:e]
        # indicator is generated on-chip from wdst16 (iota + is_equal);
        # -1 in padding slots matches no iota value -> zero indicator row
        wdst16 = np.full((128, totch), -1, np.int16)
        wdst16[slot[s:e], chunk_id[s:e]] = wdst_s[s:e]

        # encode gather indices per (block, half) run: [128, totch*8] int16
        gidx = np.zeros((16, totch * 8), np.int16)
        for blk in sched["blocks"]:
            for hkey in ("lo", "hi"):
                off, cnts = blk[hkey]
                gg = sum(cnts)
                if gg == 0:
                    continue
                local = idx16[off:off + gg].reshape(-1)      # [gg*128]
                gidx[:, off * 8:(off + gg) * 8] = local.reshape(-1, 16).T
        gidx = np.tile(gidx, (8, 1))

        # x^T pre-tiled: [W, 128, KO0, 128] bf16
        xs = np.zeros((CP, D_IN), np.float32)
        xs[:NPC] = x[c * NPC:(c + 1) * NPC]
        xt = np.ascontiguousarray(
            xs.reshape(W, 128, KOS[0], 128).transpose(0, 3, 2, 1)
        ).astype(ml_dtypes.bfloat16)
        # xt[m, p, ko, j] = xs[m*128 + j, ko*128 + p]

        dv = np.zeros((CP,), np.float32)
        dv[:NPC] = dinv[c * NPC:(c + 1) * NPC]
        dinv_own = np.ascontiguousarray(dv.reshape(W, 128).T)  # [128, W]
        qrow = np.zeros((1, CP), np.float32)
        qrow[0, :NPC] = np.sqrt(deg[c * NPC:(c + 1) * NPC])
        qrow = qrow.astype(ml_dtypes.bfloat16)

        per_core.append({
            "wdst16": wdst16, "gidx": gidx, "xt": xt, "dinv_own": dinv_own,
            "qrow": qrow,
        })
    return sched, per_core


def _weight_tiles(Wm, ko):
    """[K, M] weight -> [128, ko, M] with tile[p, k, m] = Wm[k*128+p, m]."""
    K, M = Wm.shape
    assert K == ko * 128
    return np.ascontiguousarray(
        Wm.reshape(ko, 128, M).transpose(1, 0, 2)
    )


def _build(sched, sim=False, unroll=1, ablate=()):
    """sim=True: single-core variant for TimelineSim — the AllGather is
    replaced by a local DMA copy (collectives aren't supported by the sim;
    their time is accounted separately). unroll>1 repeats the whole network
    to amortize dispatch overhead for timing."""
    nc = PatchedBacc("TRN2", num_devices=1 if sim else NCORES,
                     num_swdge_queues=NSWQ)
    totch = sched["totch"]

    xt_h = nc.dram_tensor("xt", [W, 128, KOS[0], 128], BF16, kind="ExternalInput")
    wdst_h = nc.dram_tensor("wdst16", [128, totch], I16, kind="ExternalInput")
    gidx_h = nc.dram_tensor("gidx", [128, totch * 8], I16, kind="ExternalInput")
    w1_h = nc.dram_tensor("w1", [128, KOS[0], H], BF16, kind="ExternalInput")
    w2_h = nc.dram_tensor("w2", [128, KOS[1], H], BF16, kind="ExternalInput")
    w3_h = nc.dram_tensor("w3", [128, KOS[2], H3], BF16, kind="ExternalInput")
    wc_h = nc.dram_tensor("wc", [128, OUT], F32, kind="ExternalInput")
    # biases as single rows: folded into the PSUM chain via rank-1 matmuls
    # (q[dst] x b[feat] with q = sqrt(deg), so post-scaling by dinv restores b)
    b1_h = nc.dram_tensor("b1r", [1, H], BF16, kind="ExternalInput")
    b2_h = nc.dram_tensor("b2r", [1, H], BF16, kind="ExternalInput")
    b3_h = nc.dram_tensor("b3r", [1, H3], BF16, kind="ExternalInput")
    bc_h = nc.dram_tensor("bcr", [1, OUT], BF16, kind="ExternalInput")
    q_h = nc.dram_tensor("qrow", [1, CP], BF16, kind="ExternalInput")
    dinv_h = nc.dram_tensor("dinv", [128, W], F32, kind="ExternalInput")
    y_h = nc.dram_tensor("y", [CP, OUT], F32, kind="ExternalOutput")

    ag_in_a = [
        nc.dram_tensor(f"ag_ina{l}", [RA, DOUTS[l]], TDTS[l], kind="Internal")
        for l in range(3)
    ]
    ag_in_b = [
        nc.dram_tensor(f"ag_inb{l}", [RB, DOUTS[l]], TDTS[l], kind="Internal")
        for l in range(3)
    ]
    h_sha = [
        nc.dram_tensor(f"h_sha{l}", [TBLA, DOUTS[l]], TDTS[l], kind="Internal",
                       addr_space="Shared")
        for l in range(3)
    ]
    h_shb = [
        nc.dram_tensor(f"h_shb{l}", [TBLB, DOUTS[l]], TDTS[l], kind="Internal",
                       addr_space="Shared")
        for l in range(3)
    ]

    with tile.TileContext(nc) as tc:
        with (
            tc.tile_pool(name="persist", bufs=1) as persist,
            tc.tile_pool(name="lhs", bufs=3) as lhs_pool,
            tc.tile_pool(name="gath", bufs=GBUFS) as gath_pool,
            tc.tile_pool(name="indp", bufs=IBUFS) as ind_pool,
            tc.tile_pool(name="post", bufs=3) as post_pool,
            tc.tile_pool(name="lpsum", bufs=2, space="PSUM") as lin_psum,
            tc.tile_pool(name="apsum", bufs=4, space="PSUM") as agg_psum,
            tc.tile_pool(name="mpsum", bufs=2, space="PSUM") as misc_psum,
        ):
            idx_sb = persist.tile([128, totch * 8], I16, tag="idx", name="idx")
            nc.sync.dma_start(idx_sb[:], gidx_h[:])
            wdst_sb = persist.tile([128, totch], I16, tag="wdst", name="wdst")
            nc.sync.dma_start(wdst_sb[:], wdst_h[:])
            iota_sb = persist.tile([128, 128], I16, tag="iota", name="iota")
            nc.gpsimd.iota(iota_sb[:], pattern=[[1, 128]], base=0,
                           channel_multiplier=0)
            dinv_sb = persist.tile([128, W], F32, tag="dinv", name="dinv")
            nc.sync.dma_start(dinv_sb[:], dinv_h[:])
            w_sb = [
                persist.tile([128, KOS[0], H], BF16, tag="w1", name="w1"),
                persist.tile([128, KOS[1], H], BF16, tag="w2", name="w2"),
                persist.tile([128, KOS[2], H3], BF16, tag="w3", name="w3"),
            ]
            nc.sync.dma_start(w_sb[0][:], w1_h[:])
            nc.sync.dma_start(w_sb[1][:], w2_h[:])
            nc.sync.dma_start(w_sb[2][:], w3_h[:])
            wc_sb = persist.tile([128, OUT], F32, tag="wc", name="wc")
            nc.sync.dma_start(wc_sb[:], wc_h[:])
            b_sb = [
                persist.tile([1, H], BF16, tag="b1", name="b1"),
                persist.tile([1, H], BF16, tag="b2", name="b2"),
                persist.tile([1, H3], BF16, tag="b3", name="b3"),
            ]
            nc.sync.dma_start(b_sb[0][:], b1_h[:])
            nc.sync.dma_start(b_sb[1][:], b2_h[:])
            nc.sync.dma_start(b_sb[2][:], b3_h[:])
            bc_sb = persist.tile([1, OUT], BF16, tag="bc", name="bc")
            nc.sync.dma_start(bc_sb[:], bc_h[:])
            q_sb = persist.tile([1, CP], BF16, tag="qrow", name="qrow")
            nc.sync.dma_start(q_sb[:], q_h[:])
            ones_sb = persist.tile([1, 128], BF16, tag="ones", name="ones")
            nc.gpsimd.memset(ones_sb[:], 1.0)

            id_bf = persist.tile([128, 128], BF16, tag="idbf", name="idbf")
            make_identity(nc, id_bf[:])
            id_f32 = persist.tile([128, 128], F32, tag="idf32", name="idf32")
            make_identity(nc, id_f32[:])

            # h2own ping-pongs by global layer parity: lin(gl+1) writes one
            # buffer while agg(gl)'s self-term still reads the other
            h2own2 = [
                persist.tile([128, W, H], BF16, tag="h2own0", name="h2own0"),
                persist.tile([128, W, H], BF16, tag="h2own1", name="h2own1"),
            ]
            hT = persist.tile([128, W, 2, 128], BF16, tag="hT", name="hT")
            out_sb = persist.tile([128, W, OUT], F32, tag="out", name="out")

            gq = [0]  # round-robin SWDGE queue for gathers
            GL = 3 * unroll

            def do_lin(gl, m):
                l = gl % 3
                dout = DOUTS[l]
                h2own = h2own2[gl % 2]
                if l == 0:
                    lt = lhs_pool.tile([128, KOS[0], 128], BF16, tag="xt", name="xt")
                    nc.sync.dma_start(lt[:], xt_h[m])
                    lhsT = lt
                else:
                    lhsT = hT[:, m]
                ps = lin_psum.tile([128, dout], F32, tag="lin", name="lin")
                for k in range(KOS[l]):
                    nc.tensor.matmul(
                        ps[:], lhsT[:, k, :], w_sb[l][:, k, :],
                        start=(k == 0), stop=(k == KOS[l] - 1),
                    )
                nc.scalar.mul(h2own[:, m, :dout], ps[:], dinv_sb[:, m:m + 1])
                if TDTS[l] == BF16:
                    h2tbl = h2own[:, m, :dout]
                else:
                    # second read of the same PSUM tile, converting to the
                    # fp8 message-table dtype
                    h8 = post_pool.tile([128, dout], TDTS[l], tag="h8", name="h8")
                    nc.scalar.mul(h8[:], ps[:], dinv_sb[:, m:m + 1])
                    h2tbl = h8[:]
                if m < MA:
                    nc.sync.dma_start(ag_in_a[l][m * 128:(m + 1) * 128, :], h2tbl)
                else:
                    nc.sync.dma_start(
                        ag_in_b[l][(m - MA) * 128:(m - MA + 1) * 128, :], h2tbl
                    )

            def issue_ag(gl, half):
                l = gl % 3
                src_t = ag_in_a[l] if half == 0 else ag_in_b[l]
                dst_t = h_sha[l] if half == 0 else h_shb[l]
                rows = RA if half == 0 else RB
                if sim:
                    # replicate into every rank slot: wrong data but finite,
                    # so the interpreter's NaN/race checks stay meaningful
                    for r in range(NCORES):
                        nc.sync.dma_start(
                            dst_t[r * rows:(r + 1) * rows, :], src_t[:]
                        )
                elif "noag" in ablate:
                    nc.sync.dma_start(dst_t[0:rows, :], src_t[:])
                elif "noag2" in ablate:
                    pass
                else:
                    nc.gpsimd.collective_compute(
                        "AllGather",
                        mybir.AluOpType.bypass,
                        replica_groups=[list(range(NCORES))],
                        ins=[src_t[:]],
                        outs=[dst_t[:]],
                    )

            # prologue: layer 0 lin + AGs of the first unroll
            for m in range(W):
                do_lin(0, m)
                if m == MA - 1:
                    issue_ag(0, 0)
                elif m == W - 1:
                    issue_ag(0, 1)

            for gl in range(GL):
                l = gl % 3
                dout = DOUTS[l]
                h2own = h2own2[gl % 2]
                # cross-layer fusion: lin windows of gl+1 become ready as this
                # layer's aggregation completes windows in block order; the
                # next layer's AllGathers then overlap this layer's gathers.
                fuse = gl + 1 < GL
                lin_cursor = 0
                pend_ag = []  # (gl, half, ready_bi): AG issues deferred
                              # AG_DELAY blocks past their lins so the
                              # trigger's sem wait is resolved by the time Q7
                              # reaches it (no desc-gen head-block stall)
                for bi, blk in enumerate(sched["blocks"]):
                    tiles = {}
                    for hkey, tbl_t in (("lo", h_sha[l]), ("hi", h_shb[l])):
                        off, cnts = blk[hkey]
                        gg = sum(cnts)
                        if gg == 0 or "nogather" in ablate:
                            tiles[hkey] = None
                            continue
                        gt = gath_pool.tile([128, gg, dout], TDTS[l], tag=f"g{hkey}", name=f"g{hkey}")
                        nc.gpsimd.dma_gather(
                            gt[:], tbl_t[:],
                            idx_sb[:, off * 8:(off + gg) * 8],
                            128 * gg, 128 * gg, dout,
                            single_packet=SINGLE_PACKET,
                            queue_num=gq[0] % NSWQ,
                        )
                        gq[0] += 1
                        tiles[hkey] = (gt, off)
                    while pend_ag and pend_ag[0][2] <= bi:
                        ag_gl, ag_half, _ = pend_ag.pop(0)
                        issue_ag(ag_gl, ag_half)
                    o0 = blk["lo"][0]
                    gtot = sum(blk["lo"][1]) + sum(blk["hi"][1])
                    if "noind" not in ablate:
                        it = ind_pool.tile([128, gtot, 128], F8, tag="ind", name="ind")
                        nc.vector.tensor_tensor(
                            it[:],
                            iota_sb[:, None, :].to_broadcast([128, gtot, 128]),
                            wdst_sb[:, o0:o0 + gtot, None].to_broadcast(
                                [128, gtot, 128]
                            ),
                            op=mybir.AluOpType.is_equal,
                        )
                    for wi, w in enumerate(blk["windows"]):
                        ps = agg_psum.tile([128, dout], F32, tag="agg", name="agg")
                        mms = []
                        for hkey in ("lo", "hi"):
                            if tiles[hkey] is None or "noindmm" in ablate:
                                continue
                            gt, off = tiles[hkey]
                            cnts = blk[hkey][1]
                            gstart = sum(cnts[:wi])
                            for g in range(gstart, gstart + cnts[wi]):
                                mms.append((gt, off, g))
                        for i, (gt, off, g) in enumerate(mms):
                            nc.tensor.matmul(
                                ps[:],
                                it[:, off - o0 + g, :],
                                gt[:, g, :],
                                start=(i == 0), stop=False,
                            )
                        # self term: += I @ h''own
                        nc.tensor.matmul(
                            ps[:], id_bf[:], h2own[:, w, :dout],
                            start=(len(mms) == 0), stop=False,
                        )
                        # bias fold: += q[dst] (x) b[feat]; post-scale by dinv
                        # turns it back into b
                        nc.tensor.matmul(
                            ps[:], q_sb[:, w * 128:(w + 1) * 128],
                            b_sb[l][:, :dout], start=False, stop=True,
                        )
                        if l < 2:
                            relu = post_pool.tile([128, dout], BF16, tag="relu", name="relu")
                            nc.scalar.activation(
                                relu[:], ps[:], mybir.ActivationFunctionType.Relu,
                                scale=dinv_sb[:, w:w + 1],
                            )
                            for k in range(dout // 128):
                                tp = misc_psum.tile([128, 128], BF16, tag="tp", name="tp")
                                nc.tensor.transpose(
                                    tp[:], relu[:, k * 128:(k + 1) * 128], id_bf[:]
                                )
                                nc.vector.tensor_copy(hT[:, w, k, :], tp[:])
                        else:
                            h3f = post_pool.tile([128, H3], F32, tag="h3f", name="h3f")
                            nc.scalar.activation(
                                h3f[:], ps[:], mybir.ActivationFunctionType.Relu,
                                scale=dinv_sb[:, w:w + 1],
                            )
                            tpf = misc_psum.tile([128, 128], F32, tag="tp", name="tp")
                            nc.tensor.transpose(tpf[:], h3f[:], id_f32[:])
                            h3T = post_pool.tile([128, 128], F32, tag="h3T", name="h3T")
                            nc.vector.tensor_copy(h3T[:], tpf[:])
                            p9 = misc_psum.tile([128, OUT], F32, tag="tp", name="tp")
                            nc.tensor.matmul(
                                p9[:], h3T[:], wc_sb[:], start=True, stop=False
                            )
                            nc.tensor.matmul(
                                p9[:], ones_sb[:], bc_sb[:], start=False, stop=True
                            )
                            nc.vector.tensor_copy(out_sb[:, w, :], p9[:])
                    if fuse and FUSE:
                        done_w = blk["windows"][-1] + 1
                        if done_w >= MA and lin_cursor < MA:
                            for m in range(lin_cursor, MA):
                                do_lin(gl + 1, m)
                            lin_cursor = MA
                            pend_ag.append((gl + 1, 0, bi + AG_DELAY))
                        if done_w >= W and lin_cursor < W:
                            for m in range(lin_cursor, W):
                                do_lin(gl + 1, m)
                            lin_cursor = W
                            pend_ag.append((gl + 1, 1, 0))
                for ag_gl, ag_half, _ in pend_ag:
                    issue_ag(ag_gl, ag_half)
                pend_ag = []
                if fuse and not FUSE:
                    # sequential structure: phase A of the next layer runs
                    # after this layer's aggregation, AGs fire mid-lin
                    for m in range(W):
                        do_lin(gl + 1, m)
                        if m == MA - 1:
                            issue_ag(gl + 1, 0)
                        elif m == W - 1:
                            issue_ag(gl + 1, 1)
                if l == 2:
                    nc.sync.dma_start(
                        y_h[:].rearrange("(w p) o -> p w o", p=128), out_sb[:]
                    )
    nc.compile()
    return nc


def _pjrt_prepare(nc, in_maps, k_execs=1):
    """Like bass2jax.run_bass_via_pjrt, but returns a re-executable runner
    with device-resident inputs, for wall-clock timing. With k_execs>1 the
    jitted program invokes the NEFF k times (serialized on-device), so
    (t(k) - t(1)) / (k - 1) estimates one NEFF execution."""
    import jax
    from jax.sharding import Mesh, PartitionSpec, NamedSharding
    from jax.experimental.shard_map import shard_map
    from concourse import bass2jax

    bass2jax.install_neuronx_cc_hook()
    n_cores = len(in_maps)
    partition_name = (
        nc.partition_id_tensor.name if nc.partition_id_tensor else None
    )
    in_names, out_names, out_avals, zero_outs = [], [], [], []
    for alloc in nc.m.functions[0].allocations:
        if not isinstance(alloc, mybir.MemoryLocationSet):
            continue
        name = alloc.memorylocations[0].name
        if alloc.kind == "ExternalInput":
            if name != partition_name:
                in_names.append(name)
        elif alloc.kind == "ExternalOutput":
            import jax.core
            out_names.append(name)
            aval = jax.core.ShapedArray(
                tuple(alloc.tensor_shape), mybir.dt.np(alloc.dtype)
            )
            out_avals.append(aval)
            zero_outs.append(np.zeros(aval.shape, aval.dtype))
    n_params = len(in_names)
    n_outs = len(out_names)
    in_names = in_names + out_names
    if partition_name is not None:
        in_names.append(partition_name)
    donate = tuple(range(n_params, n_params + n_outs * k_execs))

    def _body(*args):
        ins = list(args[:n_params])
        all_outs = []
        for k in range(k_execs):
            operands = ins + list(
                args[n_params + k * n_outs:n_params + (k + 1) * n_outs]
            )
            if partition_name is not None:
                operands.append(bass2jax.partition_id_tensor())
            outs = bass2jax._bass_exec_p.bind(
                *operands,
                out_avals=tuple(out_avals),
                in_names=tuple(in_names),
                out_names=tuple(out_names),
                lowering_input_output_aliases=(),
                sim_require_finite=True,
                sim_require_nnan=True,
                nc=nc,
            )
            all_outs.extend(list(outs))
        return tuple(all_outs)

    devices = jax.devices()[:n_cores]
    mesh = Mesh(np.asarray(devices), ("core",))
    sharded = jax.jit(
        shard_map(
            _body, mesh=mesh,
            in_specs=(PartitionSpec("core"),) * (n_params + n_outs * k_execs),
            out_specs=(PartitionSpec("core"),) * (n_outs * k_execs),
            check_rep=False,
        ),
        donate_argnums=donate, keep_unused=True,
    )
    sh = NamedSharding(mesh, PartitionSpec("core"))
    concat_in = [
        np.concatenate([np.asarray(in_maps[c][nm]) for c in range(n_cores)], axis=0)
        for nm in in_names[:n_params]
    ]
    dev_in = [jax.device_put(a, sh) for a in concat_in]
    jax.block_until_ready(dev_in)

    def run_once():
        zeros = [
            jax.device_put(np.zeros((n_cores * z.shape[0], *z.shape[1:]), z.dtype), sh)
            for _ in range(k_execs)
            for z in zero_outs
        ]
        jax.block_until_ready(zeros)
        import time
        t0 = time.perf_counter()
        outs = sharded(*dev_in, *zeros)
        jax.block_until_ready(outs)
        t1 = time.perf_counter()
        results = [
            {nm: np.asarray(outs[i]).reshape(n_cores, *out_avals[i].shape)[c]
             for i, nm in enumerate(out_names)}
            for c in range(n_cores)
        ]
        return results, t1 - t0

    return run_once


_CACHE = {}


def _input_maps(inputs, per_core):
    W1 = np.asarray(inputs["W1"], np.float32)
    W2 = np.asarray(inputs["W2"], np.float32)
    W3 = np.asarray(inputs["W3"], np.float32)
    wc = np.concatenate(
        [np.asarray(inputs["We"]), np.asarray(inputs["Wh"]),
         np.asarray(inputs["Wg"])], axis=1
    ).astype(np.float32)
    bc = np.concatenate(
        [np.asarray(inputs["be"]), np.asarray(inputs["bh"]),
         np.asarray(inputs["bg"])], axis=0
    ).astype(np.float32)
    shared = {
        "w1": _weight_tiles(W1, KOS[0]).astype(ml_dtypes.bfloat16),
        "w2": _weight_tiles(W2, KOS[1]).astype(ml_dtypes.bfloat16),
        "w3": _weight_tiles(W3, KOS[2]).astype(ml_dtypes.bfloat16),
        "wc": wc,
        "b1r": np.asarray(inputs["b1"], ml_dtypes.bfloat16)[None, :],
        "b2r": np.asarray(inputs["b2"], ml_dtypes.bfloat16)[None, :],
        "b3r": np.asarray(inputs["b3"], ml_dtypes.bfloat16)[None, :],
        "bcr": bc[None, :].astype(ml_dtypes.bfloat16),
    }
    return [
        {**shared, "xt": pc["xt"], "wdst16": pc["wdst16"], "gidx": pc["gidx"],
         "dinv": pc["dinv_own"], "qrow": pc["qrow"]}
        for pc in per_core
    ]


def _run(inputs, trace=False):
    x = np.asarray(inputs["x"], np.float32)
    edge_index = np.asarray(inputs["edge_index"])
    W1 = np.asarray(inputs["W1"], np.float32)
    W2 = np.asarray(inputs["W2"], np.float32)
    W3 = np.asarray(inputs["W3"], np.float32)
    We = np.asarray(inputs["We"], np.float32)
    Wh = np.asarray(inputs["Wh"], np.float32)
    Wg = np.asarray(inputs["Wg"], np.float32)
    b1 = np.asarray(inputs["b1"], np.float32)
    b2 = np.asarray(inputs["b2"], np.float32)
    b3 = np.asarray(inputs["b3"], np.float32)
    be = np.asarray(inputs["be"], np.float32)
    bh = np.asarray(inputs["bh"], np.float32)
    bg = np.asarray(inputs["bg"], np.float32)

    sched, per_core = _preprocess(x, edge_index)

    key = tuple(int(c) for c in sched["ch"].reshape(-1))
    if key not in _CACHE:
        _CACHE[key] = _build(sched)
    nc = _CACHE[key]

    in_maps = _input_maps(inputs, per_core)

    runner = _pjrt_prepare(nc, in_maps)
    results, dt = runner()
    out = np.empty((N, OUT), np.float32)
    for c in range(NCORES):
        out[c * NPC:(c + 1) * NPC] = results[c]["y"][:NPC]
    return out, {"runner": runner, "first_wall_s": dt, "nc": nc,
                 "in_maps": in_maps}


def _baseline_runner():
    """Tiny kernel through the same path: measures dispatch overhead."""
    nc = PatchedBacc("TRN2", num_devices=NCORES)
    xh = nc.dram_tensor("x", [128, 16], F32, kind="ExternalInput")
    yh = nc.dram_tensor("y", [CP, OUT], F32, kind="ExternalOutput")
    with tile.TileContext(nc) as tc:
        with tc.tile_pool(name="p", bufs=1) as pool:
            t = pool.tile([128, 16], F32, name="t")
            nc.sync.dma_start(t[:], xh[:])
            nc.sync.dma_start(yh[:128, :OUT], t[:, :OUT])
    nc.compile()
    in_maps = [{"x": np.zeros((128, 16), np.float32)} for _ in range(NCORES)]
    return _pjrt_prepare(nc, in_maps)


def kernel(**inputs) -> np.ndarray:
    out, _ = _run(inputs, trace=False)
    return out



# revision 36
# speedup vs baseline: 1.6152x; 1.6152x over previous
"""GCN (3-layer, 3-head) Trainium2 kernel, node-sharded across 8 NeuronCores.

Strategy (per core c of 8):
  - Core c owns nodes [c*6250, (c+1)*6250), padded to 6272 = 49*128.
  - Per layer: lin on own nodes (PE), scale by dinv -> h'' (bf16), AllGather
    h'' into a shared [50176, D] table, then aggregate own-dst edges:
    gather h''[src] rows via dma_gather + 0/1-indicator matmuls accumulating
    per-128-dst-window PSUM tiles; self term added via identity matmul;
    post-scale by dinv[dst], add bias, relu.
  - Edge norms are folded: norm_edge = dinv[src]*dinv[dst] becomes pre-scale
    (h'' = h' * dinv) + post-scale (out *= dinv[dst]); indicator is exact 0/1.
  - Indicators are generated on-chip (DVE iota+is_equal from a compact
    [128, totch] wdst int16 input; -1 pads match nothing) instead of being
    streamed from HBM (-43MB/core); dinv scalings run on the ACT engine to
    keep DVE headroom for indicator generation.
  - Edges are sorted by (dst-window, src-half, src-row) on the host: the
    src-row secondary sort makes gather descriptor addresses monotone per
    call (HBM locality). The shared table is split into two halves (A:
    own-rows [0,3200), B: [3200,6272) of every core) so gather indices fit
    int16 and the two AllGathers pipeline: AG_A fires mid-lin and the A-half
    gathers overlap AG_B's transfer.
  - Message tables for layers 1-2 are fp8e4m3 (halves gather + AllGather
    bytes; end-to-end rel-err ~4.4e-3 vs the 2e-2 gate). Layer 3 stays bf16
    because its 128-wide rows would be 128B in fp8, below the 256B
    dma_gather row minimum. The self-term path stays bf16 throughout.
  - Biases fold into the aggregation PSUM chain as rank-1 matmuls
    (q[dst] x b[feat], q = sqrt(deg)), so the post-path is a single fused
    scale+relu activation.
  - dma_gather descriptor generation is spread over all 4 SWDGE queues.
"""

import numpy as np
import ml_dtypes

import concourse.bass as bass
import concourse.bacc as bacc
import concourse.mybir as mybir
import concourse.tile as tile
from concourse.bass_utils import run_bass_kernel_spmd
from concourse.masks import make_identity

NCORES = 8
N = 50000
E = 800000
D_IN = 512
H = 256
H3 = 128
OUT = 9
NPC = N // NCORES          # 6250 nodes per core
W = 49                     # dst windows of 128 (49*128 = 6272)
CP = W * 128               # padded nodes per core = 6272
TBL = NCORES * CP          # padded gather-table rows = 50176
RA = 3200                  # rows-per-core in table A (25 tiles of 128)
RB = CP - RA               # rows-per-core in table B (24 tiles)
MA = RA // 128             # 25 m-tiles in A
TBLA = NCORES * RA         # 25600 (< 32767, int16-safe)
TBLB = NCORES * RB         # 24576
BLK = 3                    # dst windows per gather block
NSWQ = 4                   # SWDGE queues: parallel Q7 descriptor generation
GBUFS = 4                  # gather tile double-buffering depth
IBUFS = 2                  # indicator tile buffering depth
SINGLE_PACKET = False      # dma_gather single_packet mode
AG_DELAY = 2               # blocks between a fused AG's lins and its trigger
FUSE = False               # fuse lin(l+1)+AG(l+1) into agg(l)'s block loop;
                           # measured slightly worse than the sequential
                           # structure (AG cost is HBM-bandwidth contention,
                           # not exposable latency), so off by default
DOUTS = (H, H, H3)         # per-layer output widths
KOS = (D_IN // 128, H // 128, H // 128)  # per-layer contraction subtiles

BF16 = mybir.dt.bfloat16
F32 = mybir.dt.float32
F8 = mybir.dt.float8e4
I16 = mybir.dt.int16

# message-table dtype per layer: fp8 halves gather+AllGather bytes; L3 must
# stay bf16 (fp8 rows would be 128B < the 256B dma_gather row minimum).
# measured end-to-end rel-err with L1+L2 fp8 is ~4e-3 (gate 2e-2).
TDTS = (F8, F8, BF16)


def split_excess_waits(nc, max_waits=1):
    """This container's walrus rejects >1 sync wait per instruction: move
    excess waits onto same-engine NOPs inserted immediately before."""
    n_split = 0
    for f in nc.m.functions:
        for bb in f.blocks:
            new_insts = []
            for ins in bb.instructions:
                si = ins.sync_info
                if si is not None and si.on_wait and len(si.on_wait) > max_waits:
                    waits = list(si.on_wait)
                    excess, keep = waits[:-max_waits], waits[-max_waits:]
                    for i in range(0, len(excess), max_waits):
                        nop = mybir.InstNoOp(
                            name=f"{ins.name}-ws{n_split}",
                            engine=ins.engine,
                            sync_info=mybir.SyncInfo(
                                on_wait=excess[i:i + max_waits], on_update=[]
                            ),
                            bass_nofuse=True,
                        )
                        n_split += 1
                        nc.register_instruction(nop)
                        new_insts.append(nop)
                    si.on_wait = keep
                new_insts.append(ins)
            bb.instructions = new_insts
    return n_split


class PatchedBacc(bacc.Bacc):
    def compile(self):
        super().compile()
        split_excess_waits(self)


def _schedule(counts):
    """counts: [NCORES, W, 2] per-(core,window,half) edge counts.
    Returns the static chunk schedule shared by all cores."""
    ch = -(-counts.max(axis=0) // 128)          # [W, 2] chunks per (window, half)
    blocks = []
    off = 0
    for b0 in range(0, W, BLK):
        ws = list(range(b0, min(b0 + BLK, W)))
        lo_counts = [int(ch[w, 0]) for w in ws]
        hi_counts = [int(ch[w, 1]) for w in ws]
        lo = (off, lo_counts)
        off += sum(lo_counts)
        hi = (off, hi_counts)
        off += sum(hi_counts)
        blocks.append({"windows": ws, "lo": lo, "hi": hi})
    return {"ch": ch, "blocks": blocks, "totch": off}


def _chunk_base(sched):
    """chunk id of the first chunk of each (window, half)."""
    base = np.zeros((W, 2), np.int64)
    for blk in sched["blocks"]:
        for half, key in ((0, "lo"), (1, "hi")):
            off, cnts = blk[key]
            for w, cnt in zip(blk["windows"], cnts):
                base[w, half] = off
                off += cnt
    return base


def _preprocess(x, edge_index):
    src = edge_index[0].astype(np.int64)
    dst = edge_index[1].astype(np.int64)
    deg = np.bincount(dst, minlength=N).astype(np.float32) + 1.0
    dinv = 1.0 / np.sqrt(deg)

    oc = dst // NPC
    ldst = dst - oc * NPC
    win = ldst >> 7
    wdst = ldst & 127
    g = src // NPC
    r = src % NPC
    half = (r >= RA).astype(np.int64)
    rel = np.where(half == 0, g * RA + r, g * RB + (r - RA)).astype(np.int16)

    key = (oc * W + win) * 2 + half
    counts = np.bincount(key, minlength=NCORES * W * 2).reshape(NCORES, W, 2)
    sched = _schedule(counts)
    totch = sched["totch"]
    cbase = _chunk_base(sched)

    # secondary sort by table address (rel) within each group: gather
    # descriptors then read monotonically increasing addresses -> better
    # HBM row locality on the 512B random reads
    order = np.lexsort((rel, key))
    key_s = key[order]
    rel_s = rel[order]
    wdst_s = wdst[order]
    win_s = win[order]
    half_s = half[order]
    oc_s = oc[order]
    # offset of each edge within its (core, window, half) group
    group_start = np.zeros(NCORES * W * 2 + 1, np.int64)
    np.cumsum(counts.reshape(-1), out=group_start[1:])
    off_in_group = np.arange(E) - group_start[key_s]
    chunk_id = cbase[win_s, half_s] + (off_in_group >> 7)
    slot = off_in_group & 127

    per_core = []
    core_edge_start = np.searchsorted(oc_s, np.arange(NCORES + 1))
    for c in range(NCORES):
        s, e = core_edge_start[c], core_edge_start[c + 1]
        idx16 = np.zeros((totch, 128), np.int16)
        idx16[chunk_id[s:e], slot[s:e]] = rel_s[s:e]
        # indicator is generated on-chip from wdst16 (iota + is_equal);
        # -1 in padding slots matches no iota value -> zero indicator row
        wdst16 = np.full((128, totch), -1, np.int16)
        wdst16[slot[s:e], chunk_id[s:e]] = wdst_s[s:e]

        # encode gather indices per (block, half) run: [128, totch*8] int16
        gidx = np.zeros((16, totch * 8), np.int16)
        for blk in sched["blocks"]:
            for hkey in ("lo", "hi"):
                off, cnts = blk[hkey]
                gg = sum(cnts)
                if gg == 0:
                    continue
                local = idx16[off:off + gg].reshape(-1)      # [gg*128]
                gidx[:, off * 8:(off + gg) * 8] = local.reshape(-1, 16).T
        gidx = np.tile(gidx, (8, 1))

        # x^T pre-tiled: [W, 128, KO0, 128] bf16
        xs = np.zeros((CP, D_IN), np.float32)
        xs[:NPC] = x[c * NPC:(c + 1) * NPC]
        xt = np.ascontiguousarray(
            xs.reshape(W, 128, KOS[0], 128).transpose(0, 3, 2, 1)
        ).astype(ml_dtypes.bfloat16)
        # xt[m, p, ko, j] = xs[m*128 + j, ko*128 + p]

        dv = np.zeros((CP,), np.float32)
        dv[:NPC] = dinv[c * NPC:(c + 1) * NPC]
        dinv_own = np.ascontiguousarray(dv.reshape(W, 128).T)  # [128, W]
        qrow = np.zeros((1, CP), np.float32)
        qrow[0, :NPC] = np.sqrt(deg[c * NPC:(c + 1) * NPC])
        qrow = qrow.astype(ml_dtypes.bfloat16)

        per_core.append({
            "wdst16": wdst16, "gidx": gidx, "xt": xt, "dinv_own": dinv_own,
            "qrow": qrow,
        })
    return sched, per_core


def _weight_tiles(Wm, ko):
    """[K, M] weight -> [128, ko, M] with tile[p, k, m] = Wm[k*128+p, m]."""
    K, M = Wm.shape
    assert K == ko * 128
    return np.ascontiguousarray(
        Wm.reshape(ko, 128, M).transpose(1, 0, 2)
    )


def _build(sched, sim=False, unroll=1, ablate=()):
    """sim=True: single-core variant for TimelineSim — the AllGather is
    replaced by a local DMA copy (collectives aren't supported by the sim;
    their time is accounted separately). unroll>1 repeats the whole network
    to amortize dispatch overhead for timing."""
    nc = PatchedBacc("TRN2", num_devices=1 if sim else NCORES,
                     num_swdge_queues=NSWQ)
    totch = sched["totch"]

    xt_h = nc.dram_tensor("xt", [W, 128, KOS[0], 128], BF16, kind="ExternalInput")
    wdst_h = nc.dram_tensor("wdst16", [128, totch], I16, kind="ExternalInput")
    gidx_h = nc.dram_tensor("gidx", [128, totch * 8], I16, kind="ExternalInput")
    w1_h = nc.dram_tensor("w1", [128, KOS[0], H], BF16, kind="ExternalInput")
    w2_h = nc.dram_tensor("w2", [128, KOS[1], H], BF16, kind="ExternalInput")
    w3_h = nc.dram_tensor("w3", [128, KOS[2], H3], BF16, kind="ExternalInput")
    wc_h = nc.dram_tensor("wc", [128, OUT], F32, kind="ExternalInput")
    # biases as single rows: folded into the PSUM chain via rank-1 matmuls
    # (q[dst] x b[feat] with q = sqrt(deg), so post-scaling by dinv restores b)
    b1_h = nc.dram_tensor("b1r", [1, H], BF16, kind="ExternalInput")
    b2_h = nc.dram_tensor("b2r", [1, H], BF16, kind="ExternalInput")
    b3_h = nc.dram_tensor("b3r", [1, H3], BF16, kind="ExternalInput")
    bc_h = nc.dram_tensor("bcr", [1, OUT], BF16, kind="ExternalInput")
    q_h = nc.dram_tensor("qrow", [1, CP], BF16, kind="ExternalInput")
    dinv_h = nc.dram_tensor("dinv", [128, W], F32, kind="ExternalInput")
    y_h = nc.dram_tensor("y", [CP, OUT], F32, kind="ExternalOutput")

    ag_in_a = [
        nc.dram_tensor(f"ag_ina{l}", [RA, DOUTS[l]], TDTS[l], kind="Internal")
        for l in range(3)
    ]
    ag_in_b = [
        nc.dram_tensor(f"ag_inb{l}", [RB, DOUTS[l]], TDTS[l], kind="Internal")
        for l in range(3)
    ]
    h_sha = [
        nc.dram_tensor(f"h_sha{l}", [TBLA, DOUTS[l]], TDTS[l], kind="Internal",
                       addr_space="Shared")
        for l in range(3)
    ]
    h_shb = [
        nc.dram_tensor(f"h_shb{l}", [TBLB, DOUTS[l]], TDTS[l], kind="Internal",
                       addr_space="Shared")
        for l in range(3)
    ]

    with tile.TileContext(nc) as tc:
        with (
            tc.tile_pool(name="persist", bufs=1) as persist,
            tc.tile_pool(name="lhs", bufs=3) as lhs_pool,
            tc.tile_pool(name="gath", bufs=GBUFS) as gath_pool,
            tc.tile_pool(name="indp", bufs=IBUFS) as ind_pool,
            tc.tile_pool(name="post", bufs=3) as post_pool,
            tc.tile_pool(name="lpsum", bufs=2, space="PSUM") as lin_psum,
            tc.tile_pool(name="apsum", bufs=4, space="PSUM") as agg_psum,
            tc.tile_pool(name="mpsum", bufs=2, space="PSUM") as misc_psum,
        ):
            idx_sb = persist.tile([128, totch * 8], I16, tag="idx", name="idx")
            nc.sync.dma_start(idx_sb[:], gidx_h[:])
            wdst_sb = persist.tile([128, totch], I16, tag="wdst", name="wdst")
            nc.sync.dma_start(wdst_sb[:], wdst_h[:])
            iota_sb = persist.tile([128, 128], I16, tag="iota", name="iota")
            nc.gpsimd.iota(iota_sb[:], pattern=[[1, 128]], base=0,
                           channel_multiplier=0)
            dinv_sb = persist.tile([128, W], F32, tag="dinv", name="dinv")
            nc.sync.dma_start(dinv_sb[:], dinv_h[:])
            w_sb = [
                persist.tile([128, KOS[0], H], BF16, tag="w1", name="w1"),
                persist.tile([128, KOS[1], H], BF16, tag="w2", name="w2"),
                persist.tile([128, KOS[2], H3], BF16, tag="w3", name="w3"),
            ]
            nc.sync.dma_start(w_sb[0][:], w1_h[:])
            nc.sync.dma_start(w_sb[1][:], w2_h[:])
            nc.sync.dma_start(w_sb[2][:], w3_h[:])
            wc_sb = persist.tile([128, OUT], F32, tag="wc", name="wc")
            nc.sync.dma_start(wc_sb[:], wc_h[:])
            b_sb = [
                persist.tile([1, H], BF16, tag="b1", name="b1"),
                persist.tile([1, H], BF16, tag="b2", name="b2"),
                persist.tile([1, H3], BF16, tag="b3", name="b3"),
            ]
            nc.sync.dma_start(b_sb[0][:], b1_h[:])
            nc.sync.dma_start(b_sb[1][:], b2_h[:])
            nc.sync.dma_start(b_sb[2][:], b3_h[:])
            bc_sb = persist.tile([1, OUT], BF16, tag="bc", name="bc")
            nc.sync.dma_start(bc_sb[:], bc_h[:])
            q_sb = persist.tile([1, CP], BF16, tag="qrow", name="qrow")
            nc.sync.dma_start(q_sb[:], q_h[:])
            ones_sb = persist.tile([1, 128], BF16, tag="ones", name="ones")
            nc.gpsimd.memset(ones_sb[:], 1.0)

            id_bf = persist.tile([128, 128], BF16, tag="idbf", name="idbf")
            make_identity(nc, id_bf[:])
            id_f32 = persist.tile([128, 128], F32, tag="idf32", name="idf32")
            make_identity(nc, id_f32[:])

            # h2own ping-pongs by global layer parity: lin(gl+1) writes one
            # buffer while agg(gl)'s self-term still reads the other
            h2own2 = [
                persist.tile([128, W, H], BF16, tag="h2own0", name="h2own0"),
                persist.tile([128, W, H], BF16, tag="h2own1", name="h2own1"),
            ]
            hT = persist.tile([128, W, 2, 128], BF16, tag="hT", name="hT")
            out_sb = persist.tile([128, W, OUT], F32, tag="out", name="out")

            gq = [0]  # round-robin SWDGE queue for gathers
            GL = 3 * unroll

            def do_lin(gl, m):
                l = gl % 3
                dout = DOUTS[l]
                h2own = h2own2[gl % 2]
                if l == 0:
                    lt = lhs_pool.tile([128, KOS[0], 128], BF16, tag="xt", name="xt")
                    nc.sync.dma_start(lt[:], xt_h[m])
                    lhsT = lt
                else:
                    lhsT = hT[:, m]
                ps = lin_psum.tile([128, dout], F32, tag="lin", name="lin")
                for k in range(KOS[l]):
                    nc.tensor.matmul(
                        ps[:], lhsT[:, k, :], w_sb[l][:, k, :],
                        start=(k == 0), stop=(k == KOS[l] - 1),
                    )
                nc.scalar.mul(h2own[:, m, :dout], ps[:], dinv_sb[:, m:m + 1])
                if TDTS[l] == BF16:
                    h2tbl = h2own[:, m, :dout]
                else:
                    # second read of the same PSUM tile, converting to the
                    # fp8 message-table dtype
                    h8 = post_pool.tile([128, dout], TDTS[l], tag="h8", name="h8")
                    nc.scalar.mul(h8[:], ps[:], dinv_sb[:, m:m + 1])
                    h2tbl = h8[:]
                if m < MA:
                    nc.sync.dma_start(ag_in_a[l][m * 128:(m + 1) * 128, :], h2tbl)
                else:
                    nc.sync.dma_start(
                        ag_in_b[l][(m - MA) * 128:(m - MA + 1) * 128, :], h2tbl
                    )

            def issue_ag(gl, half):
                l = gl % 3
                src_t = ag_in_a[l] if half == 0 else ag_in_b[l]
                dst_t = h_sha[l] if half == 0 else h_shb[l]
                rows = RA if half == 0 else RB
                if sim:
                    # replicate into every rank slot: wrong data but finite,
                    # so the interpreter's NaN/race checks stay meaningful
                    for r in range(NCORES):
                        nc.sync.dma_start(
                            dst_t[r * rows:(r + 1) * rows, :], src_t[:]
                        )
                elif "noag" in ablate:
                    nc.sync.dma_start(dst_t[0:rows, :], src_t[:])
                elif "noag2" in ablate:
                    pass
                else:
                    nc.gpsimd.collective_compute(
                        "AllGather",
                        mybir.AluOpType.bypass,
                        replica_groups=[list(range(NCORES))],
                        ins=[src_t[:]],
                        outs=[dst_t[:]],
                    )

            # prologue: layer 0 lin + AGs of the first unroll
            for m in range(W):
                do_lin(0, m)
                if m == MA - 1:
                    issue_ag(0, 0)
                elif m == W - 1:
                    issue_ag(0, 1)

            for gl in range(GL):
                l = gl % 3
                dout = DOUTS[l]
                h2own = h2own2[gl % 2]
                # cross-layer fusion: lin windows of gl+1 become ready as this
                # layer's aggregation completes windows in block order; the
                # next layer's AllGathers then overlap this layer's gathers.
                fuse = gl + 1 < GL
                lin_cursor = 0
                pend_ag = []  # (gl, half, ready_bi): AG issues deferred
                              # AG_DELAY blocks past their lins so the
                              # trigger's sem wait is resolved by the time Q7
                              # reaches it (no desc-gen head-block stall)
                for bi, blk in enumerate(sched["blocks"]):
                    tiles = {}
                    for hkey, tbl_t in (("lo", h_sha[l]), ("hi", h_shb[l])):
                        off, cnts = blk[hkey]
                        gg = sum(cnts)
                        if gg == 0 or "nogather" in ablate:
                            tiles[hkey] = None
                            continue
                        gt = gath_pool.tile([128, gg, dout], TDTS[l], tag=f"g{hkey}", name=f"g{hkey}")
                        nc.gpsimd.dma_gather(
                            gt[:], tbl_t[:],
                            idx_sb[:, off * 8:(off + gg) * 8],
                            128 * gg, 128 * gg, dout,
                            single_packet=SINGLE_PACKET,
                            queue_num=gq[0] % NSWQ,
                        )
                        gq[0] += 1
                        tiles[hkey] = (gt, off)
                    while pend_ag and pend_ag[0][2] <= bi:
                        ag_gl, ag_half, _ = pend_ag.pop(0)
                        issue_ag(ag_gl, ag_half)
                    o0 = blk["lo"][0]
                    gtot = sum(blk["lo"][1]) + sum(blk["hi"][1])
                    if "noind" not in ablate:
                        it = ind_pool.tile([128, gtot, 128], F8, tag="ind", name="ind")
                        nc.vector.tensor_tensor(
                            it[:],
                            iota_sb[:, None, :].to_broadcast([128, gtot, 128]),
                            wdst_sb[:, o0:o0 + gtot, None].to_broadcast(
                                [128, gtot, 128]
                            ),
                            op=mybir.AluOpType.is_equal,
                        )
                    for wi, w in enumerate(blk["windows"]):
                        ps = agg_psum.tile([128, dout], F32, tag="agg", name="agg")
                        mms = []
                        for hkey in ("lo", "hi"):
                            if tiles[hkey] is None or "noindmm" in ablate:
                                continue
                            gt, off = tiles[hkey]
                            cnts = blk[hkey][1]
                            gstart = sum(cnts[:wi])
                            for g in range(gstart, gstart + cnts[wi]):
                                mms.append((gt, off, g))
                        for i, (gt, off, g) in enumerate(mms):
                            nc.tensor.matmul(
                                ps[:],
                                it[:, off - o0 + g, :],
                                gt[:, g, :],
                                start=(i == 0), stop=False,
                            )
                        # self term: += I @ h''own
                        nc.tensor.matmul(
                            ps[:], id_bf[:], h2own[:, w, :dout],
                            start=(len(mms) == 0), stop=False,
                        )
                        # bias fold: += q[dst] (x) b[feat]; post-scale by dinv
                        # turns it back into b
                        nc.tensor.matmul(
                            ps[:], q_sb[:, w * 128:(w + 1) * 128],
                            b_sb[l][:, :dout], start=False, stop=True,
                        )
                        if l < 2:
                            relu = post_pool.tile([128, dout], BF16, tag="relu", name="relu")
                            nc.scalar.activation(
                                relu[:], ps[:], mybir.ActivationFunctionType.Relu,
                                scale=dinv_sb[:, w:w + 1],
                            )
                            for k in range(dout // 128):
                                tp = misc_psum.tile([128, 128], BF16, tag="tp", name="tp")
                                nc.tensor.transpose(
                                    tp[:], relu[:, k * 128:(k + 1) * 128], id_bf[:]
                                )
                                nc.vector.tensor_copy(hT[:, w, k, :], tp[:])
                        else:
                            h3f = post_pool.tile([128, H3], F32, tag="h3f", name="h3f")
                            nc.scalar.activation(
                                h3f[:], ps[:], mybir.ActivationFunctionType.Relu,
                                scale=dinv_sb[:, w:w + 1],
                            )
                            tpf = misc_psum.tile([128, 128], F32, tag="tp", name="tp")
                            nc.tensor.transpose(tpf[:], h3f[:], id_f32[:])
                            h3T = post_pool.tile([128, 128], F32, tag="h3T", name="h3T")
                            nc.vector.tensor_copy(h3T[:], tpf[:])
                            p9 = misc_psum.tile([128, OUT], F32, tag="tp", name="tp")
                            nc.tensor.matmul(
                                p9[:], h3T[:], wc_sb[:], start=True, stop=False
                            )
                            nc.tensor.matmul(
                                p9[:], ones_sb[:], bc_sb[:], start=False, stop=True
                            )
                            nc.vector.tensor_copy(out_sb[:, w, :], p9[:])
                    if fuse and FUSE:
                        done_w = blk["windows"][-1] + 1
                        if done_w >= MA and lin_cursor < MA:
                            for m in range(lin_cursor, MA):
                                do_lin(gl + 1, m)
                            lin_cursor = MA
                            pend_ag.append((gl + 1, 0, bi + AG_DELAY))
                        if done_w >= W and lin_cursor < W:
                            for m in range(lin_cursor, W):
                                do_lin(gl + 1, m)
                            lin_cursor = W
                            pend_ag.append((gl + 1, 1, 0))
                for ag_gl, ag_half, _ in pend_ag:
                    issue_ag(ag_gl, ag_half)
                pend_ag = []
                if fuse and not FUSE:
                    # sequential structure: phase A of the next layer runs
                    # after this layer's aggregation, AGs fire mid-lin
                    for m in range(W):
                        do_lin(gl + 1, m)
                        if m == MA - 1:
                            issue_ag(gl + 1, 0)
                        elif m == W - 1:
                            issue_ag(gl + 1, 1)
                if l == 2:
                    nc.sync.dma_start(
                        y_h[:].rearrange("(w p) o -> p w o", p=128), out_sb[:]
                    )
    nc.compile()
    return nc


def _pjrt_prepare(nc, in_maps, k_execs=1):
    """Like bass2jax.run_bass_via_pjrt, but returns a re-executable runner
    with device-resident inputs, for wall-clock timing. With k_execs>1 the
    jitted program invokes the NEFF k times (serialized on-device), so
    (t(k) - t(1)) / (k - 1) estimates one NEFF execution."""
    import jax
    from jax.sharding import Mesh, PartitionSpec, NamedSharding
    from jax.experimental.shard_map import shard_map
    from concourse import bass2jax

    bass2jax.install_neuronx_cc_hook()
    n_cores = len(in_maps)
    partition_name = (
        nc.partition_id_tensor.name if nc.partition_id_tensor else None
    )
    in_names, out_names, out_avals, zero_outs = [], [], [], []
    for alloc in nc.m.functions[0].allocations:
        if not isinstance(alloc, mybir.MemoryLocationSet):
            continue
        name = alloc.memorylocations[0].name
        if alloc.kind == "ExternalInput":
            if name != partition_name:
                in_names.append(name)
        elif alloc.kind == "ExternalOutput":
            import jax.core
            out_names.append(name)
            aval = jax.core.ShapedArray(
                tuple(alloc.tensor_shape), mybir.dt.np(alloc.dtype)
            )
            out_avals.append(aval)
            zero_outs.append(np.zeros(aval.shape, aval.dtype))
    n_params = len(in_names)
    n_outs = len(out_names)
    in_names = in_names + out_names
    if partition_name is not None:
        in_names.append(partition_name)
    donate = tuple(range(n_params, n_params + n_outs * k_execs))

    def _body(*args):
        ins = list(args[:n_params])
        all_outs = []
        for k in range(k_execs):
            operands = ins + list(
                args[n_params + k * n_outs:n_params + (k + 1) * n_outs]
            )
            if partition_name is not None:
                operands.append(bass2jax.partition_id_tensor())
            outs = bass2jax._bass_exec_p.bind(
                *operands,
                out_avals=tuple(out_avals),
                in_names=tuple(in_names),
                out_names=tuple(out_names),
                lowering_input_output_aliases=(),
                sim_require_finite=True,
                sim_require_nnan=True,
                nc=nc,
            )
            all_outs.extend(list(outs))
        return tuple(all_outs)

    devices = jax.devices()[:n_cores]
    mesh = Mesh(np.asarray(devices), ("core",))
    sharded = jax.jit(
        shard_map(
            _body, mesh=mesh,
            in_specs=(PartitionSpec("core"),) * (n_params + n_outs * k_execs),
            out_specs=(PartitionSpec("core"),) * (n_outs * k_execs),
            check_rep=False,
        ),
        donate_argnums=donate, keep_unused=True,
    )
    sh = NamedSharding(mesh, PartitionSpec("core"))
    concat_in = [
        np.concatenate([np.asarray(in_maps[c][nm]) for c in range(n_cores)], axis=0)
        for nm in in_names[:n_params]
    ]
    dev_in = [jax.device_put(a, sh) for a in concat_in]
    jax.block_until_ready(dev_in)

    def run_once():
        zeros = [
            jax.device_put(np.zeros((n_cores * z.shape[0], *z.shape[1:]), z.dtype), sh)
            for _ in range(k_execs)
            for z in zero_outs
        ]
        jax.block_until_ready(zeros)
        import time
        t0 = time.perf_counter()
        outs = sharded(*dev_in, *zeros)
        jax.block_until_ready(outs)
        t1 = time.perf_counter()
        results = [
            {nm: np.asarray(outs[i]).reshape(n_cores, *out_avals[i].shape)[c]
             for i, nm in enumerate(out_names)}
            for c in range(n_cores)
        ]
        return results, t1 - t0

    return run_once


_CACHE = {}


def _input_maps(inputs, per_core):
    W1 = np.asarray(inputs["W1"], np.float32)
    W2 = np.asarray(inputs["W2"], np.float32)
    W3 = np.asarray(inputs["W3"], np.float32)
    wc = np.concatenate(
        [np.asarray(inputs["We"]), np.asarray(inputs["Wh"]),
         np.asarray(inputs["Wg"])], axis=1
    ).astype(np.float32)
    bc = np.concatenate(
        [np.asarray(inputs["be"]), np.asarray(inputs["bh"]),
         np.asarray(inputs["bg"])], axis=0
    ).astype(np.float32)
    shared = {
        "w1": _weight_tiles(W1, KOS[0]).astype(ml_dtypes.bfloat16),
        "w2": _weight_tiles(W2, KOS[1]).astype(ml_dtypes.bfloat16),
        "w3": _weight_tiles(W3, KOS[2]).astype(ml_dtypes.bfloat16),
        "wc": wc,
        "b1r": np.asarray(inputs["b1"], ml_dtypes.bfloat16)[None, :],
        "b2r": np.asarray(inputs["b2"], ml_dtypes.bfloat16)[None, :],
        "b3r": np.asarray(inputs["b3"], ml_dtypes.bfloat16)[None, :],
        "bcr": bc[None, :].astype(ml_dtypes.bfloat16),
    }
    return [
        {**shared, "xt": pc["xt"], "wdst16": pc["wdst16"], "gidx": pc["gidx"],
         "dinv": pc["dinv_own"], "qrow": pc["qrow"]}
        for pc in per_core
    ]


def _run(inputs, trace=False):
    x = np.asarray(inputs["x"], np.float32)
    edge_index = np.asarray(inputs["edge_index"])
    W1 = np.asarray(inputs["W1"], np.float32)
    W2 = np.asarray(inputs["W2"], np.float32)
    W3 = np.asarray(inputs["W3"], np.float32)
    We = np.asarray(inputs["We"], np.float32)
    Wh = np.asarray(inputs["Wh"], np.float32)
    Wg = np.asarray(inputs["Wg"], np.float32)
    b1 = np.asarray(inputs["b1"], np.float32)
    b2 = np.asarray(inputs["b2"], np.float32)
    b3 = np.asarray(inputs["b3"], np.float32)
    be = np.asarray(inputs["be"], np.float32)
    bh = np.asarray(inputs["bh"], np.float32)
    bg = np.asarray(inputs["bg"], np.float32)

    sched, per_core = _preprocess(x, edge_index)

    key = tuple(int(c) for c in sched["ch"].reshape(-1))
    if key not in _CACHE:
        _CACHE[key] = _build(sched)
    nc = _CACHE[key]

    in_maps = _input_maps(inputs, per_core)

    runner = _pjrt_prepare(nc, in_maps)
    results, dt = runner()
    out = np.empty((N, OUT), np.float32)
    for c in range(NCORES):
        out[c * NPC:(c + 1) * NPC] = results[c]["y"][:NPC]
    return out, {"runner": runner, "first_wall_s": dt, "nc": nc,
                 "in_maps": in_maps}


def _baseline_runner():
    """Tiny kernel through the same path: measures dispatch overhead."""
    nc = PatchedBacc("TRN2", num_devices=NCORES)
    xh = nc.dram_tensor("x", [128, 16], F32, kind="ExternalInput")
    yh = nc.dram_tensor("y", [CP, OUT], F32, kind="ExternalOutput")
    with tile.TileContext(nc) as tc:
        with tc.tile_pool(name="p", bufs=1) as pool:
            t = pool.tile([128, 16], F32, name="t")
            nc.sync.dma_start(t[:], xh[:])
            nc.sync.dma_start(yh[:128, :OUT], t[:, :OUT])
    nc.compile()
    in_maps = [{"x": np.zeros((128, 16), np.float32)} for _ in range(NCORES)]
    return _pjrt_prepare(nc, in_maps)


def kernel(**inputs) -> np.ndarray:
    out, _ = _run(inputs, trace=False)
    return out

